# revision 3
# baseline (speedup 1.0000x reference)
"""Distributed multi-head attention block for 8 Trainium2 NeuronCores.

Problem: y = out_proj(softmax(Q K^T / sqrt(dk)) V) for
x [4, 2048, 2048], 16 heads, dk=128, torch-Linear weights (y = x @ W^T).

Sharding: 2-way data parallel over batch pairs x 4-way tensor parallel over
head groups (4 heads / 512 features per group). Core c handles batches
[2p, 2p+1] (p = c // 4) and heads [4g .. 4g+3] (g = c % 4). Each core
computes a partial output y_c = Ot_g^T @ woT_g for its head group; the host
sums the 4 group partials per batch pair.

Layout strategy (all matmuls contract over the SBUF partition dim):
  - host pre-transposes x -> xT [d, s] and weights -> wT [d, e] and converts
    everything to bf16, so no on-device transposes are needed and every
    matmul runs at 1 cycle/row with ~2x cheaper PE stationary loads than
    f32r (measured: 245-274 ns vs 295-317 ns per [128,512] matmul).
  - Q, K are produced head-transposed (Qt/Kt [dk, s]) and spilled to DRAM;
    V is produced natural ([s, dk]) and kept SBUF-resident for the whole
    kernel (v_all, bf16) - PV consumes its [k,dk] chunks as stationary
    directly, no V spill/readback DMA at all.
  - scores are computed transposed, S^T [k, q] = Kt^T-slice . Qt, so the
    PV matmul (out^T [dk,q] = V^T P^T) consumes exp(S^T) directly with no
    transposes anywhere.
  - softmax denominators: exp tiles are pair/quad-summed on the (otherwise
    idle) DVE and Pool engines, and only the 4 quad tiles per q-block
    stream through an all-ones stationary matmul (PE cost 4x lower than
    accumulating all 16 k-chunks on PE; every row of the result identical
    so the reciprocal broadcast is free). 1/sqrt(dk) is folded into wq on
    the host; softmax-max subtraction is skipped (scores ~ N(0,1), exp is
    safe well within bf16/fp32 range).
  - projection phase runs on s-block PAIRS so each Q/K weight stationary
    serves two consecutive matmuls (halves PE stationary reloads), and the
    initial weight/x DMAs are chunked per-128-row so the first matmul chain
    starts after ~1 MiB instead of ~16 MiB of DMA.
  - the (b=0, h=0/1) Qt/Kt readbacks are issued mid-projection (right after
    batch 0's spills), so attention starts with no DMA gap at the phase
    transition.
"""

import sys

if "/opt/trn_rl_repo" not in sys.path:
    sys.path.insert(0, "/opt/trn_rl_repo")

import numpy as np

import concourse.bacc as bacc
import concourse.mybir as mybir
import concourse.tile as tile
from concourse.bass_utils import run_bass_kernel_spmd

F32 = mybir.dt.float32
F32R = mybir.dt.float32r
BF16 = mybir.dt.bfloat16

B = 4  # batch
S = 2048  # sequence length
D = 2048  # model dim
H = 16  # heads
DK = 128  # head dim

NCORES = 8
BPC = 2  # batches per core (data parallel over pairs)
GROUPS = 4  # head groups (tensor parallel)
HPG = H // GROUPS  # heads per group = 4
EG = HPG * DK  # per-group projection width = 512
SL = BPC * S  # local sequence rows per core = 4096

SB = 512  # s-block width for the projection phase
QB = 512  # q-block width for the attention phase
FB = 512  # f-block width for the output projection
DC = D // 128  # contraction chunks for the projections = 16
KC = S // 128  # k chunks per (b, h) attention = 16
NCH = SL // 128  # v_all s-chunks = 32

_CACHE: dict = {}
XBUFS = 4  # x s-block tiles (2 live + 2 prefetch)
PSF_BUFS = 4  # fine scores-psum slots
QKVBUFS = 2  # attention Qt/Kt readback prefetch depth
POOL_EVERY = 3  # every POOL_EVERY-th pair-add goes to the Pool engine


def _build(nrep: int = 1, pdt=BF16, adt=BF16):
    """Build the per-core Bass program (identical on all 8 cores)."""
    nc = bacc.Bacc("TRN2", target_bir_lowering=False, debug=False, num_devices=NCORES)

    xT = nc.dram_tensor("xT", [D, SL], pdt, kind="ExternalInput").ap()
    wqT = nc.dram_tensor("wqT", [D, EG], pdt, kind="ExternalInput").ap()
    wkT = nc.dram_tensor("wkT", [D, EG], pdt, kind="ExternalInput").ap()
    wvT = nc.dram_tensor("wvT", [D, EG], pdt, kind="ExternalInput").ap()
    woT = nc.dram_tensor("woT", [EG, D], adt, kind="ExternalInput").ap()
    y = nc.dram_tensor("y", [SL, D], F32, kind="ExternalOutput").ap()

    with tile.TileContext(nc) as tc:
        with tc.tile_pool(name="dram", bufs=1, space="DRAM") as dram:
            qt = dram.tile([EG, SL], adt)  # Qt (head-transposed, pre-scaled)
            kt = dram.tile([EG, SL], adt)  # Kt (head-transposed)

            for _ in range(nrep):
                with (
                    tc.tile_pool(name="persist", bufs=1) as pers,
                    tc.tile_pool(name="qkv", bufs=QKVBUFS) as qkvpool,
                ):
                    v_all = pers.tile([128, NCH, EG], adt, tag="v_all")
                    pre = _emit_projections(
                        nc, tc, xT, wqT, wkT, wvT, qt, kt, v_all, qkvpool, pdt, adt
                    )
                    _emit_attention(
                        nc, tc, woT, y, qt, kt, v_all, qkvpool, pre, adt
                    )

    nc.compile()
    return nc


def _prefetch_qkt(nc, qkvpool, qt, kt, b, h, adt):
    """Issue the Qt/Kt readback DMAs for head (b, h)."""
    qt_s = qkvpool.tile([128, S], adt, tag="qts")
    kt_s = qkvpool.tile([128, S], adt, tag="kts")
    e0, s0 = h * 128, b * S
    nc.sync.dma_start(qt_s[:], qt[e0 : e0 + 128, s0 : s0 + S])
    nc.sync.dma_start(kt_s[:], kt[e0 : e0 + 128, s0 : s0 + S])
    return qt_s, kt_s


def _emit_projections(nc, tc, xT, wqT, wkT, wvT, qt, kt, v_all, qkvpool, pdt, adt):
    """Phase 1: Qt/Kt [EG, SL] spilled to DRAM, V -> v_all SBUF (bf16).

    s-blocks are processed in pairs so each Q/K weight stationary slice
    serves two back-to-back matmuls. Returns prefetched (b0, h0/h1)
    Qt/Kt readback tiles for the attention phase.
    """
    with (
        tc.tile_pool(name="wproj", bufs=1) as wpool,
        tc.tile_pool(name="xin", bufs=XBUFS) as xpool,
        tc.tile_pool(name="pevict", bufs=4) as epool,
        tc.tile_pool(name="pproj", bufs=4, space="PSUM") as ppool,
    ):
        wq_s = wpool.tile([128, DC, EG], pdt, tag="wq")
        wk_s = wpool.tile([128, DC, EG], pdt, tag="wk")
        wv_s = wpool.tile([128, DC, EG], pdt, tag="wv")

        # First s-block pair + weights, chunked per-dc and interleaved so the
        # first Q accumulation chain (needs all dc of x pair + wq) completes
        # after ~19 us of DMA instead of waiting for all 16 MiB.
        xa0 = xpool.tile([128, DC, SB], pdt, tag="xts", name="xa0")
        xb0 = xpool.tile([128, DC, SB], pdt, tag="xts", name="xb0")
        for dc in range(DC):
            r = slice(dc * 128, (dc + 1) * 128)
            nc.sync.dma_start(xa0[:, dc, :], xT[r, 0:SB])
            nc.sync.dma_start(xb0[:, dc, :], xT[r, SB : 2 * SB])
            nc.sync.dma_start(wq_s[:, dc, :], wqT[r, :])
        for dc in range(DC):
            nc.sync.dma_start(wk_s[:, dc, :], wkT[dc * 128 : (dc + 1) * 128, :])
        for dc in range(DC):
            nc.sync.dma_start(wv_s[:, dc, :], wvT[dc * 128 : (dc + 1) * 128, :])

        pre = []
        for sp in range(SL // (2 * SB)):  # 4 s-block pairs
            if sp == 0:
                xa, xb = xa0, xb0
            else:
                xa = xpool.tile([128, DC, SB], pdt, tag="xts", name=f"xa{sp}")
                xb = xpool.tile([128, DC, SB], pdt, tag="xts", name=f"xb{sp}")
                for xt_, sb in ((xa, 2 * sp), (xb, 2 * sp + 1)):
                    nc.sync.dma_start(
                        xt_[:],
                        xT[:, sb * SB : (sb + 1) * SB].rearrange(
                            "(dc p) s -> p dc s", p=128
                        ),
                    )
            # Qt / Kt: out[e-chunk 128, s 512] accumulated over d; each
            # stationary w slice feeds both s-blocks of the pair.
            for w_s, dst in ((wq_s, qt), (wk_s, kt)):
                for ec in range(EG // 128):
                    pa = ppool.tile([128, SB], F32, tag="pp", name="pa")
                    pb = ppool.tile([128, SB], F32, tag="pp", name="pb")
                    for dc in range(DC):
                        st = w_s[:, dc, ec * 128 : (ec + 1) * 128]
                        nc.tensor.matmul(
                            pa[:], st, xa[:, dc, :], start=(dc == 0), stop=(dc == DC - 1)
                        )
                        nc.tensor.matmul(
                            pb[:], st, xb[:, dc, :], start=(dc == 0), stop=(dc == DC - 1)
                        )
                    for ps, sb in ((pa, 2 * sp), (pb, 2 * sp + 1)):
                        ev = epool.tile([128, SB], adt, tag="ev")
                        nc.vector.tensor_copy(out=ev[:], in_=ps[:])
                        nc.sync.dma_start(
                            dst[ec * 128 : (ec + 1) * 128, sb * SB : (sb + 1) * SB],
                            ev[:],
                        )
            # V: out[s-chunk 128, e 512] accumulated over d (roles swapped),
            # written straight into the SBUF-resident v_all (f32 -> bf16).
            for xt_, sb in ((xa, 2 * sp), (xb, 2 * sp + 1)):
                for sc in range(SB // 128):
                    ps = ppool.tile([128, EG], F32, tag="pv")
                    for dc in range(DC):
                        nc.tensor.matmul(
                            ps[:],
                            xt_[:, dc, sc * 128 : (sc + 1) * 128],
                            wv_s[:, dc, :],
                            start=(dc == 0),
                            stop=(dc == DC - 1),
                        )
                    nc.vector.tensor_copy(out=v_all[:, sb * 4 + sc, :], in_=ps[:])
            if sp == 1:
                # batch 0 fully spilled: start its first readbacks now so
                # attention begins with zero transition gap.
                pre.append(_prefetch_qkt(nc, qkvpool, qt, kt, 0, 0, adt))
                pre.append(_prefetch_qkt(nc, qkvpool, qt, kt, 0, 1, adt))
        return pre


def _emit_attention(nc, tc, woT, y, qt, kt, v_all, qkvpool, pre, adt):
    """Phase 2: per (b, h) flash-style attention + per-b output projection."""
    with (
        tc.tile_pool(name="watt", bufs=1) as wpool,
        tc.tile_pool(name="ptile", bufs=8) as ptpool,
        tc.tile_pool(name="pairs", bufs=4) as papool,
        tc.tile_pool(name="quads", bufs=10) as qupool,
        tc.tile_pool(name="rdt", bufs=4) as rdpool,
        tc.tile_pool(name="ott", bufs=2 * HPG) as otpool,
        tc.tile_pool(name="yev", bufs=4) as ypool,
        tc.tile_pool(name="psatt", bufs=PSF_BUFS, space="PSUM") as pspool,
        tc.tile_pool(name="psacc", bufs=2, space="PSUM") as popool,
        tc.tile_pool(name="psden", bufs=2, space="PSUM") as pdpool,
    ):
        wo_s = wpool.tile([128, HPG, D], adt, tag="wo")
        nc.sync.dma_start(wo_s[:], woT.rearrange("(hc p) f -> p hc f", p=128))
        ones_f = wpool.tile([128, 128], F32, tag="ones_f")
        nc.vector.memset(ones_f[:], 1.0)
        ones = wpool.tile([128, 128], adt, tag="ones")
        nc.vector.tensor_copy(out=ones[:], in_=ones_f[:])

        add_ctr = [0]

        def tensor_add(out, a, b_):
            # split the denominator adds between DVE and Pool so neither
            # engine sits on the attention critical path
            add_ctr[0] += 1
            eng = nc.gpsimd if add_ctr[0] % POOL_EVERY == 0 else nc.vector
            eng.tensor_add(out[:], a[:], b_[:])

        for b in range(BPC):
            s0 = b * S
            ot_tiles = []
            for h in range(HPG):
                if b == 0 and h < len(pre):
                    qt_s, kt_s = pre[h]
                else:
                    qt_s, kt_s = _prefetch_qkt(nc, qkvpool, qt, kt, b, h, adt)
                ot = otpool.tile([128, S], adt, tag="ot")
                for qp in range(S // (2 * QB)):
                    qbs = (2 * qp, 2 * qp + 1)
                    ps_o = [
                        popool.tile([128, QB], F32, tag="po", name=f"ps_o{i}")
                        for i in range(2)
                    ]
                    ps_d = [
                        pdpool.tile([128, QB], F32, tag="pd", name=f"ps_d{i}")
                        for i in range(2)
                    ]
                    # software-pipelined: S^T tiles + exp for chunk kc+1 are
                    # emitted before the PV matmuls of chunk kc, so PE never
                    # stalls on the ACT exp.
                    pts = [[None, None] for _ in range(KC)]
                    pend = [None, None]  # pending pair tile per half
                    quads = [[], []]

                    def score_exp(kc, qbs=qbs, qt_s=qt_s, kt_s=kt_s, pts=pts):
                        for i, qb in enumerate(qbs):
                            ps_f = pspool.tile(
                                [128, QB], F32, tag="psf", name=f"ps_f{i}"
                            )
                            nc.tensor.matmul(
                                ps_f[:],
                                kt_s[:, kc * 128 : (kc + 1) * 128],
                                qt_s[:, qb * QB : (qb + 1) * QB],
                                start=True,
                                stop=True,
                            )
                            ph = ptpool.tile(
                                [128, QB], adt, tag="ptf", name=f"pt_f{i}"
                            )
                            nc.scalar.activation(
                                ph[:], ps_f[:], mybir.ActivationFunctionType.Exp
                            )
                            pts[kc][i] = ph

                    def emit_sums(kc, pts=pts, pend=pend, quads=quads):
                        # pair/quad-sum the exp tiles on DVE/Pool; only the
                        # quads stream through PE for the denominator.
                        if kc % 2 != 1:
                            return
                        for i in range(2):
                            pr = papool.tile([128, QB], adt, tag="pa")
                            tensor_add(pr, pts[kc - 1][i], pts[kc][i])
                            if kc % 4 == 1:
                                pend[i] = pr
                            else:
                                qd = qupool.tile([128, QB], adt, tag="qd")
                                tensor_add(qd, pend[i], pr)
                                quads[i].append(qd)

                    score_exp(0)
                    for kc in range(KC):
                        if kc + 1 < KC:
                            score_exp(kc + 1)
                        for i in range(2):
                            nc.tensor.matmul(
                                ps_o[i][:],
                                v_all[:, b * KC + kc, h * 128 : (h + 1) * 128],
                                pts[kc][i][:],
                                start=(kc == 0),
                                stop=(kc == KC - 1),
                            )
                        emit_sums(kc)
                    # denominator: 4 quads per half through the all-ones
                    # stationary (every output row identical)
                    for i in range(2):
                        for j, qd in enumerate(quads[i]):
                            nc.tensor.matmul(
                                ps_d[i][:], ones[:], qd[:], start=(j == 0), stop=(j == 3)
                            )
                    for i, qb in enumerate(qbs):
                        rd = rdpool.tile([128, QB], F32, tag="rd")
                        nc.vector.reciprocal(rd[:], ps_d[i][:])
                        nc.vector.tensor_mul(
                            ot[:, qb * QB : (qb + 1) * QB], ps_o[i][:], rd[:]
                        )
                ot_tiles.append(ot)
            # output projection for batch b: y[s, f] += Ot_h^T . woT_h; each
            # Ot stationary slice serves two f-blocks back to back.
            for sc in range(S // 128):
                for fp in range(D // (2 * FB)):
                    ps_y = [
                        pdpool.tile([128, FB], F32, tag="pd", name=f"ps_y{i}")
                        for i in range(2)
                    ]
                    for h in range(HPG):
                        for i in range(2):
                            fb = 2 * fp + i
                            nc.tensor.matmul(
                                ps_y[i][:],
                                ot_tiles[h][:, sc * 128 : (sc + 1) * 128],
                                wo_s[:, h, fb * FB : (fb + 1) * FB],
                                start=(h == 0),
                                stop=(h == HPG - 1),
                            )
                    for i in range(2):
                        fb = 2 * fp + i
                        yt = ypool.tile([128, FB], F32, tag="yt")
                        nc.vector.tensor_copy(out=yt[:], in_=ps_y[i][:])
                        nc.sync.dma_start(
                            y[
                                s0 + sc * 128 : s0 + (sc + 1) * 128,
                                fb * FB : (fb + 1) * FB,
                            ],
                            yt[:],
                        )


def _np_dt(dt):
    return mybir.dt.np(dt)


def _prepare_in_maps(x, wq, wk, wv, wo, pdt=BF16, adt=BF16):
    x = np.ascontiguousarray(np.asarray(x, dtype=np.float32))
    wq = np.asarray(wq, dtype=np.float32)
    wk = np.asarray(wk, dtype=np.float32)
    wv = np.asarray(wv, dtype=np.float32)
    wo = np.asarray(wo, dtype=np.float32)

    np_p, np_a = _np_dt(pdt), _np_dt(adt)
    scale = np.float32(1.0 / np.sqrt(DK))
    xT_pair = [
        np.ascontiguousarray(x[2 * p : 2 * p + 2].reshape(BPC * S, D).T).astype(np_p)
        for p in range(NCORES // GROUPS)
    ]
    wqT_g, wkT_g, wvT_g, woT_g = [], [], [], []
    for g in range(GROUPS):
        eg = slice(g * EG, (g + 1) * EG)
        wqT_g.append(np.ascontiguousarray(wq[eg].T * scale).astype(np_p))
        wkT_g.append(np.ascontiguousarray(wk[eg].T).astype(np_p))
        wvT_g.append(np.ascontiguousarray(wv[eg].T).astype(np_p))
        woT_g.append(np.ascontiguousarray(wo[:, eg].T).astype(np_a))

    in_maps = []
    for c in range(NCORES):
        p, g = c // GROUPS, c % GROUPS
        in_maps.append(
            {
                "xT": xT_pair[p],
                "wqT": wqT_g[g],
                "wkT": wkT_g[g],
                "wvT": wvT_g[g],
                "woT": woT_g[g],
            }
        )
    return in_maps


def kernel(x, wq, wk, wv, wo):
    in_maps = _prepare_in_maps(x, wq, wk, wv, wo)

    if "nc" not in _CACHE:
        _CACHE["nc"] = _build()
    nc = _CACHE["nc"]

    res = run_bass_kernel_spmd(nc, in_maps, core_ids=list(range(NCORES)))

    # host gather: sum the 4 head-group partials per batch pair
    out = np.zeros((B, S, D), dtype=np.float32)
    for p in range(NCORES // GROUPS):
        acc = res.results[p * GROUPS]["y"].copy()
        for g in range(1, GROUPS):
            acc += res.results[p * GROUPS + g]["y"]
        out[2 * p : 2 * p + 2] = acc.reshape(BPC, S, D)
    return out


# revision 4
# speedup vs baseline: 1.0749x; 1.0749x over previous
"""Distributed multi-head attention block for 8 Trainium2 NeuronCores.

Problem: y = out_proj(softmax(Q K^T / sqrt(dk)) V) for
x [4, 2048, 2048], 16 heads, dk=128, torch-Linear weights (y = x @ W^T).

Sharding: 2-way data parallel over batch pairs x 4-way tensor parallel over
head groups (4 heads / 512 features per group). Core c handles batches
[2p, 2p+1] (p = c // 4) and heads [4g .. 4g+3] (g = c % 4). Each core
computes a partial output y_c = Ot_g^T @ woT_g for its head group; the host
sums the 4 group partials per batch pair.

Layout strategy (all matmuls contract over the SBUF partition dim):
  - host pre-transposes x -> xT [d, s], weights -> wT [d, e], all bf16: every
    matmul runs at 1 cycle/row with ~2x cheaper PE stationary loads than f32r
    (measured 245-274 vs 295-317 ns per [128,512] matmul on HW).
  - Q, K are produced head-transposed (Qt/Kt [dk, s]) and spilled to DRAM;
    V is produced natural ([s, dk]) and kept SBUF-resident (v_all, bf16), so
    PV consumes its [k,dk] chunks as stationary with no V DMA at all.
  - scores are computed transposed, S^T [k, q] = Kt^T-slice . Qt, so the PV
    matmul (out^T [dk,q] = V^T P^T) consumes exp(S^T) with no transposes.
  - softmax denominators: exp tiles are pair/quad-summed on the otherwise
    idle DVE, and only the 4 quad tiles per q-block-pair stream through an
    all-ones stationary matmul (4x less PE than accumulating all 16 k-chunks
    on PE; every result row identical so the reciprocal broadcast is free).
    1/sqrt(dk) is folded into wq on the host; softmax-max subtraction is
    skipped (scores ~ N(0,1), exp is safe).
  - FUSED EMISSION: the kernel is one interleaved instruction stream.
    Projection s-blocks 0-3 (batch 0) run first (with per-128-row chunked
    initial weight/x DMAs so PE starts after ~1 MiB of DMA); then batch 0's
    attention units are woven between the remaining projection chains
    (s-blocks 4-7), then batch 0's output projection is woven into batch 1's
    attention. The ACT exp stream (the attention inner-loop bottleneck,
    ~1.07 us per 1024-wide exp vs 0.85 us of PE matmuls per k-chunk) then
    overlaps dense PE work instead of stalling it.
"""

import sys

if "/opt/trn_rl_repo" not in sys.path:
    sys.path.insert(0, "/opt/trn_rl_repo")

import numpy as np

import concourse.bacc as bacc
import concourse.mybir as mybir
import concourse.tile as tile
from concourse.bass_utils import run_bass_kernel_spmd

F32 = mybir.dt.float32
F32R = mybir.dt.float32r
BF16 = mybir.dt.bfloat16

B = 4  # batch
S = 2048  # sequence length
D = 2048  # model dim
H = 16  # heads
DK = 128  # head dim

NCORES = 8
BPC = 2  # batches per core (data parallel over pairs)
GROUPS = 4  # head groups (tensor parallel)
HPG = H // GROUPS  # heads per group = 4
EG = HPG * DK  # per-group projection width = 512
SL = BPC * S  # local sequence rows per core = 4096

SB = 512  # s-block width for the projection phase
QB = 512  # q-block width for the attention phase
FB = 512  # f-block width for the output projection
DC = D // 128  # contraction chunks for the projections = 16
KC = S // 128  # k chunks per (b, h) attention = 16
NCH = SL // 128  # v_all s-chunks = 32

_CACHE: dict = {}
QKVBUFS = 2  # attention Qt/Kt readback prefetch depth


def _build(nrep: int = 1, pdt=BF16, adt=BF16):
    """Build the per-core Bass program (identical on all 8 cores)."""
    nc = bacc.Bacc("TRN2", target_bir_lowering=False, debug=False, num_devices=NCORES)

    xT = nc.dram_tensor("xT", [D, SL], pdt, kind="ExternalInput").ap()
    wqT = nc.dram_tensor("wqT", [D, EG], pdt, kind="ExternalInput").ap()
    wkT = nc.dram_tensor("wkT", [D, EG], pdt, kind="ExternalInput").ap()
    wvT = nc.dram_tensor("wvT", [D, EG], pdt, kind="ExternalInput").ap()
    woT = nc.dram_tensor("woT", [EG, D], adt, kind="ExternalInput").ap()
    y = nc.dram_tensor("y", [SL, D], F32, kind="ExternalOutput").ap()

    with tile.TileContext(nc) as tc:
        with tc.tile_pool(name="dram", bufs=1, space="DRAM") as dram:
            qt = dram.tile([EG, SL], adt)
            kt = dram.tile([EG, SL], adt)
            for _ in range(nrep):
                _emit_fused(nc, tc, xT, wqT, wkT, wvT, woT, y, qt, kt, pdt, adt)

    nc.compile()
    return nc


def _weave(primary, fill, fill_per_unit, credit=2.0):
    """Drive `primary` to exhaustion, advancing `fill` by ~fill_per_unit
    generator-steps between primary units; drain whatever remains. The
    initial credit puts a couple of fill units ahead of the first primary
    unit so its readback DMAs are covered by dense PE work."""
    alive = fill is not None
    for _ in primary:
        while alive and credit >= 1.0:
            credit -= 1.0
            if next(fill, _DONE) is _DONE:
                alive = False
        credit += fill_per_unit
    if alive:
        for _ in fill:
            pass


_DONE = object()


def _emit_fused(nc, tc, xT, wqT, wkT, wvT, woT, y, qt, kt, pdt, adt):
    with (
        tc.tile_pool(name="pers", bufs=1) as pers,
        tc.tile_pool(name="qkv", bufs=QKVBUFS) as qkvpool,
        tc.tile_pool(name="ptile", bufs=3) as ptpool,
        tc.tile_pool(name="pairs", bufs=2) as papool,
        tc.tile_pool(name="quads", bufs=2) as qupool,
        tc.tile_pool(name="rdt", bufs=2) as rdpool,
        tc.tile_pool(name="otu", bufs=2) as oupool,
        tc.tile_pool(name="ott", bufs=2 * HPG) as otpool,
        tc.tile_pool(name="yev", bufs=3) as ypool,
        tc.tile_pool(name="psatt", bufs=1, space="PSUM") as pspool,
        tc.tile_pool(name="psacc", bufs=2, space="PSUM") as popool,
        tc.tile_pool(name="psden", bufs=1, space="PSUM") as pdpool,
        tc.tile_pool(name="psproj", bufs=2, space="PSUM") as pjpool,
    ):
        v_all = pers.tile([128, NCH, EG], adt, tag="v_all")
        wo_s = pers.tile([128, HPG, D], adt, tag="wo")
        ones = pers.tile([128, 128], adt, tag="ones")
        nc.vector.memset(ones[:], 1.0)

        ot_store = {}

        # ---------------- attention generator ----------------
        def att_units(b, pre):
            ot_tiles = []
            pending = [None]  # deferred end-of-q-block tail

            def emit_tail():
                if pending[0] is None:
                    return
                ps_d, otu, ot_, qbs_ = pending[0]
                pending[0] = None
                for i, qb in enumerate(qbs_):
                    rd = rdpool.tile([128, QB], F32, tag="rd")
                    nc.vector.reciprocal(rd[:], ps_d[:, i * QB : (i + 1) * QB])
                    nc.vector.tensor_mul(
                        ot_[:, qb * QB : (qb + 1) * QB],
                        otu[:, i * QB : (i + 1) * QB],
                        rd[:],
                    )

            for h in range(HPG):
                if pre:
                    qt_s, kt_s = pre.pop(0)
                else:
                    qt_s, kt_s = _prefetch_qkt(nc, qkvpool, qt, kt, b, h, adt)
                ot = otpool.tile([128, S], adt, tag="ot")
                for qp in range(S // (2 * QB)):
                    qbs = (2 * qp, 2 * qp + 1)
                    ps_o = [
                        popool.tile([128, QB], F32, tag="po", name=f"po{i}")
                        for i in range(2)
                    ]
                    # both denominator halves in one 2-bank tile, accumulated
                    # incrementally as each quad-sum becomes ready
                    ps_d = pdpool.tile([128, 2 * QB], F32, tag="pd")
                    pts = [None] * KC
                    pend = [None]
                    nquad = [0]

                    def score_exp(kc, qt_s=qt_s, kt_s=kt_s, pts=pts, qbs=qbs):
                        # both q-block scores land in one 2-bank PSUM tile;
                        # one 1024-wide exp keeps ACT off the critical path
                        ps_s = pspool.tile([128, 2 * QB], F32, tag="psf")
                        for i, qb in enumerate(qbs):
                            nc.tensor.matmul(
                                ps_s[:, i * QB : (i + 1) * QB],
                                kt_s[:, kc * 128 : (kc + 1) * 128],
                                qt_s[:, qb * QB : (qb + 1) * QB],
                                start=True,
                                stop=True,
                            )
                        pt = ptpool.tile([128, 2 * QB], adt, tag="pt")
                        nc.scalar.activation(
                            pt[:], ps_s[:], mybir.ActivationFunctionType.Exp
                        )
                        pts[kc] = pt

                    score_exp(0)
                    score_exp(1)
                    for kc in range(KC):
                        for i in range(2):
                            nc.tensor.matmul(
                                ps_o[i][:],
                                v_all[:, b * KC + kc, h * 128 : (h + 1) * 128],
                                pts[kc][:, i * QB : (i + 1) * QB],
                                start=(kc == 0),
                                stop=(kc == KC - 1),
                            )
                        if kc == 2:
                            # previous q-block's reciprocal+rescale, deferred
                            # so its PE/DVE chain never stalls this one
                            emit_tail()
                        if kc % 2 == 1:
                            pr = papool.tile([128, 2 * QB], adt, tag="pa")
                            # batch 1 shares its attention region with the
                            # output projection (busier DVE), so route a third
                            # of its pair-adds to the otherwise idle Pool
                            eng = nc.gpsimd if (b == 1 and kc % 6 == 1) else nc.vector
                            eng.tensor_add(pr[:], pts[kc - 1][:], pts[kc][:])
                            if kc % 4 == 1:
                                pend[0] = pr
                            else:
                                qd = qupool.tile([128, 2 * QB], adt, tag="qd")
                                nc.vector.tensor_add(qd[:], pend[0][:], pr[:])
                                # stream this quad through the ones stationary
                                # now instead of batching four at qp end
                                j = nquad[0]
                                nquad[0] += 1
                                for i in range(2):
                                    nc.tensor.matmul(
                                        ps_d[:, i * QB : (i + 1) * QB],
                                        ones[:],
                                        qd[:, i * QB : (i + 1) * QB],
                                        start=(j == 0),
                                        stop=(j == 3),
                                    )
                        if kc + 2 < KC:
                            score_exp(kc + 2)
                        if kc == KC - 1:
                            # free the PV accumulators immediately with an
                            # unnormalized bf16 copy; normalization happens in
                            # the deferred tail
                            otu = oupool.tile([128, 2 * QB], adt, tag="otu")
                            for i in range(2):
                                nc.vector.tensor_copy(
                                    out=otu[:, i * QB : (i + 1) * QB], in_=ps_o[i][:]
                                )
                            pending[0] = (ps_d, otu, ot, qbs)
                        yield
                ot_tiles.append(ot)
            emit_tail()
            ot_store[b] = ot_tiles

        # ---------------- output projection generator ----------------
        def outproj_units(b, psum_tags, nfp):
            # nfp=2 reuses each Ot stationary for two f-blocks (fewer PE
            # stationary loads) but needs 4 psum slots to pipeline; nfp=1
            # pipelines within 2 slots for the region woven into attention.
            s0 = b * S
            ot_tiles = ot_store[b]
            ti = 0
            for sc in range(S // 128):
                for fp in range(D // (nfp * FB)):
                    ps_y = []
                    for i in range(nfp):
                        pool_, tag = psum_tags[ti % len(psum_tags)]
                        ti += 1
                        ps_y.append(
                            pool_.tile([128, FB], F32, tag=tag, name=f"py{i}")
                        )
                    for h in range(HPG):
                        for i in range(nfp):
                            fb = nfp * fp + i
                            nc.tensor.matmul(
                                ps_y[i][:],
                                ot_tiles[h][:, sc * 128 : (sc + 1) * 128],
                                wo_s[:, h, fb * FB : (fb + 1) * FB],
                                start=(h == 0),
                                stop=(h == HPG - 1),
                            )
                    for i in range(nfp):
                        fb = nfp * fp + i
                        yt = ypool.tile([128, FB], F32, tag="yt")
                        nc.vector.tensor_copy(out=yt[:], in_=ps_y[i][:])
                        nc.sync.dma_start(
                            y[
                                s0 + sc * 128 : s0 + (sc + 1) * 128,
                                fb * FB : (fb + 1) * FB,
                            ],
                            yt[:],
                        )
                    yield

        # ---------------- projection emission ----------------
        with (
            tc.tile_pool(name="wproj", bufs=1) as wpool,
            tc.tile_pool(name="xin", bufs=2) as xpool,
            tc.tile_pool(name="pevict", bufs=2) as epool,
        ):
            wq_s = wpool.tile([128, DC, EG], pdt, tag="wq")
            wk_s = wpool.tile([128, DC, EG], pdt, tag="wk")
            wv_s = wpool.tile([128, DC, EG], pdt, tag="wv")

            def load_x(sb, chunked):
                xt_ = xpool.tile([128, DC, SB], pdt, tag="xts", name=f"x{sb}")
                if chunked:
                    # interleave x0 and wq in dc-quad chunks (the first Q
                    # chain's inputs; coarse enough that per-DMA issue time
                    # doesn't dominate), then stream wk and wv while the Q
                    # chains compute
                    CH = 4
                    for c0 in range(0, DC, CH):
                        r = slice(c0 * 128, (c0 + CH) * 128)
                        nc.sync.dma_start(
                            xt_[:, c0 : c0 + CH, :],
                            xT[r, sb * SB : (sb + 1) * SB].rearrange(
                                "(c p) s -> p c s", p=128
                            ),
                        )
                        nc.sync.dma_start(
                            wq_s[:, c0 : c0 + CH, :],
                            wqT[r, :].rearrange("(c p) e -> p c e", p=128),
                        )
                    for w_s_, wT_ in ((wk_s, wkT), (wv_s, wvT)):
                        for c0 in range(0, DC, CH):
                            r = slice(c0 * 128, (c0 + CH) * 128)
                            nc.sync.dma_start(
                                w_s_[:, c0 : c0 + CH, :],
                                wT_[r, :].rearrange("(c p) e -> p c e", p=128),
                            )
                else:
                    nc.sync.dma_start(
                        xt_[:],
                        xT[:, sb * SB : (sb + 1) * SB].rearrange(
                            "(dc p) s -> p dc s", p=128
                        ),
                    )
                return xt_

            def proj_sblock(sb, xt_, psum_tags):
                """Q/K/V chains for one s-block; yields after each chain."""
                ti = 0
                for w_s, dst in ((wq_s, qt), (wk_s, kt)):
                    for ec in range(EG // 128):
                        pool_, tag = psum_tags[ti % len(psum_tags)]
                        ti += 1
                        ps = pool_.tile([128, SB], F32, tag=tag, name=f"pj{sb}")
                        for dc in range(DC):
                            nc.tensor.matmul(
                                ps[:],
                                w_s[:, dc, ec * 128 : (ec + 1) * 128],
                                xt_[:, dc, :],
                                start=(dc == 0),
                                stop=(dc == DC - 1),
                            )
                        ev = epool.tile([128, SB], adt, tag="ev")
                        nc.vector.tensor_copy(out=ev[:], in_=ps[:])
                        nc.sync.dma_start(
                            dst[ec * 128 : (ec + 1) * 128, sb * SB : (sb + 1) * SB],
                            ev[:],
                        )
                        yield
                for sc in range(SB // 128):
                    pool_, tag = psum_tags[ti % len(psum_tags)]
                    ti += 1
                    ps = pool_.tile([128, EG], F32, tag=tag, name=f"pv{sb}")
                    for dc in range(DC):
                        nc.tensor.matmul(
                            ps[:],
                            xt_[:, dc, sc * 128 : (sc + 1) * 128],
                            wv_s[:, dc, :],
                            start=(dc == 0),
                            stop=(dc == DC - 1),
                        )
                    nc.vector.tensor_copy(out=v_all[:, sb * 4 + sc, :], in_=ps[:])
                    yield

            def proj_range(sb_lo, sb_hi, psum_tags):
                xt_ = load_x(sb_lo, chunked=(sb_lo == 0))
                for sb in range(sb_lo, sb_hi):
                    xt_next = load_x(sb + 1, chunked=False) if sb + 1 < sb_hi else None
                    for _ in proj_sblock(sb, xt_, psum_tags):
                        yield
                    xt_ = xt_next

            # Region 1: projection of batch 0 (s-blocks 0-3), PE-dense,
            # 4-deep psum rotation through the pj+po tags (attention hasn't
            # started, so the po banks are free).
            for _ in proj_range(0, 4, [(pjpool, "pj"), (popool, "po")]):
                pass
            # batch 0 fully spilled: issue its first Qt/Kt readbacks now,
            # plus the wo load (first needed by region 3)
            pre = [
                _prefetch_qkt(nc, qkvpool, qt, kt, 0, 0, adt),
                _prefetch_qkt(nc, qkvpool, qt, kt, 0, 1, adt),
            ]
            nc.sync.dma_start(wo_s[:], woT.rearrange("(hc p) f -> p hc f", p=128))
            # Region 2: batch-1 projection (48 chains) woven between batch-0
            # attention units (128), ~0.375 chains per unit.
            _weave(
                att_units(0, pre),
                proj_range(4, 8, [(pjpool, "pj")]),
                48.0 / 128.0,
            )
        # Region 3: batch-0 output projection (64 single-f-block groups, so
        # the 2 pj psum slots pipeline) woven between batch-1 attention
        # units (128).
        _weave(att_units(1, []), outproj_units(0, [(pjpool, "pj")], 1), 64.0 / 128.0)
        # Region 4: batch-1 output projection, PE-dense tail; attention is
        # done, so rotate across 4 psum slots for clean pipelining.
        for _ in outproj_units(1, [(pjpool, "pj"), (popool, "po")], 2):
            pass


def _prefetch_qkt(nc, qkvpool, qt, kt, b, h, adt):
    qt_s = qkvpool.tile([128, S], adt, tag="qts")
    kt_s = qkvpool.tile([128, S], adt, tag="kts")
    e0, s0 = h * 128, b * S
    nc.sync.dma_start(qt_s[:], qt[e0 : e0 + 128, s0 : s0 + S])
    nc.sync.dma_start(kt_s[:], kt[e0 : e0 + 128, s0 : s0 + S])
    return qt_s, kt_s


def _np_dt(dt):
    return mybir.dt.np(dt)


def _prepare_in_maps(x, wq, wk, wv, wo, pdt=BF16, adt=BF16):
    x = np.ascontiguousarray(np.asarray(x, dtype=np.float32))
    wq = np.asarray(wq, dtype=np.float32)
    wk = np.asarray(wk, dtype=np.float32)
    wv = np.asarray(wv, dtype=np.float32)
    wo = np.asarray(wo, dtype=np.float32)

    np_p, np_a = _np_dt(pdt), _np_dt(adt)
    scale = np.float32(1.0 / np.sqrt(DK))
    xT_pair = [
        np.ascontiguousarray(x[2 * p : 2 * p + 2].reshape(BPC * S, D).T).astype(np_p)
        for p in range(NCORES // GROUPS)
    ]
    wqT_g, wkT_g, wvT_g, woT_g = [], [], [], []
    for g in range(GROUPS):
        eg = slice(g * EG, (g + 1) * EG)
        wqT_g.append(np.ascontiguousarray(wq[eg].T * scale).astype(np_p))
        wkT_g.append(np.ascontiguousarray(wk[eg].T).astype(np_p))
        wvT_g.append(np.ascontiguousarray(wv[eg].T).astype(np_p))
        woT_g.append(np.ascontiguousarray(wo[:, eg].T).astype(np_a))

    in_maps = []
    for c in range(NCORES):
        p, g = c // GROUPS, c % GROUPS
        in_maps.append(
            {
                "xT": xT_pair[p],
                "wqT": wqT_g[g],
                "wkT": wkT_g[g],
                "wvT": wvT_g[g],
                "woT": woT_g[g],
            }
        )
    return in_maps


def kernel(x, wq, wk, wv, wo):
    in_maps = _prepare_in_maps(x, wq, wk, wv, wo)

    if "nc" not in _CACHE:
        _CACHE["nc"] = _build()
    nc = _CACHE["nc"]

    res = run_bass_kernel_spmd(nc, in_maps, core_ids=list(range(NCORES)))

    out = np.zeros((B, S, D), dtype=np.float32)
    for p in range(NCORES // GROUPS):
        acc = res.results[p * GROUPS]["y"].copy()
        for g in range(1, GROUPS):
            acc += res.results[p * GROUPS + g]["y"]
        out[2 * p : 2 * p + 2] = acc.reshape(BPC, S, D)
    return out
